# revision 1
# baseline (speedup 1.0000x reference)
"""Pin2PinAttraction energy kernel for 8 TRN2 NeuronCores (Bass/Tile).

E = sum_e w_e * ((x[a_e]-x[b_e])^2 + (y[a_e]-y[b_e])^2)

Sharding: edge-parallel across the 8 cores (pairs/weights split 8 ways),
per-core partial energies reduced at the end (scalar all-reduce done on the
host after gathering the 8x128 partials).

Division of labor. This axon/PJRT stack lowers vector-indirect DMA to one
descriptor per SBUF partition (128 gathers per instruction), which makes
per-element device-side gathers of 20M random 8-byte pin rows orders of
magnitude slower than the memory roofline, and `tensor_tensor_reduce`
faults the exec unit (both probed empirically on hardware). So the
host performs only the index-dependent data *movement* — gathering
xy[a]/xy[b] rows into per-core streaming layout, no arithmetic — and the
device computes the full energy: d = va - vb, d2 = d*d, weighted sum via
free-dim reduce, fp32 accumulation across tiles.

Device per-core work: streams 2x 5MB fp16 gathered operands + 5MB fp32
weights from HBM (fp16 operand quantization contributes ~1e-7 relative
error to the energy; verified 5.8e-08 vs the fp64 reference at full size),
subtract on DVE, square on ACT (fp16 in, fp32 out), weight-multiply and
free-dim reduce on DVE, fp32 accumulators, one [128] partial out.
Measured ~35us/exec device time (repeat-slope method), memory-bound.
"""

import numpy as np
from contextlib import ExitStack

import concourse.bass as bass
import concourse.mybir as mybir
import concourse.tile as tile
from concourse import bacc
from concourse.bass_utils import run_bass_kernel_spmd

NUM_PINS = 2_000_000
NUM_PAIRS = 10_000_000
N_CORES = 8
PAIRS_PER_CORE = NUM_PAIRS // N_CORES  # 1,250,000
P = 128


def _plan(pairs_per_core):
    """Pick (T, n_tiles): n_tiles*P*T >= pairs_per_core, small padding."""
    target_tile_pairs = 125_000  # ~3MB of operand per tile
    T = max(1, target_tile_pairs // P)
    n_tiles = -(-pairs_per_core // (P * T))
    return T, n_tiles


T, N_TILES = _plan(PAIRS_PER_CORE)  # T=976, N_TILES=11
CAP = N_TILES * P * T


def build_nc(t=T, n_tiles=N_TILES, repeat=1):
    nc = bacc.Bacc(None, target_bir_lowering=False, debug=False)
    with tile.TileContext(nc) as tc:
        with tc.tile_pool(name="dram", bufs=1, space="DRAM") as dram:
            va = dram.tile([n_tiles, P, t, 2], mybir.dt.float16,
                           kind="ExternalInput", name="va", uniquify=False)
            vb = dram.tile([n_tiles, P, t, 2], mybir.dt.float16,
                           kind="ExternalInput", name="vb", uniquify=False)
            wt = dram.tile([n_tiles, P, t], mybir.dt.float32,
                           kind="ExternalInput", name="wt", uniquify=False)
            partial = dram.tile([P, 1], mybir.dt.float32,
                                kind="ExternalOutput", name="partial",
                                uniquify=False)
            _body(tc, va, vb, wt, partial, t, n_tiles, repeat)
    nc.compile()
    return nc


def _body(tc, va, vb, wt, partial, t, n_tiles, repeat=1):
    nc = tc.nc
    with ExitStack() as ctx:
        io = ctx.enter_context(tc.tile_pool(name="io", bufs=3))
        accp = ctx.enter_context(tc.tile_pool(name="accp", bufs=1))
        acc = accp.tile([P, 1], mybir.dt.float32, name="acc")
        tsum = accp.tile([P, 1], mybir.dt.float32, name="tsum")
        nc.vector.memset(acc[:], 0.0)
        for r in range(repeat):
          for i in range(n_tiles):
            ta = io.tile([P, t, 2], mybir.dt.float16, tag="ta",
                         name=f"ta{r}_{i}")
            tb = io.tile([P, t, 2], mybir.dt.float16, tag="tb",
                         name=f"tb{r}_{i}")
            sq = io.tile([P, t, 2], mybir.dt.float32, tag="sq",
                         name=f"sq{r}_{i}")
            tw = io.tile([P, t], mybir.dt.float32, tag="tw", name=f"tw{r}_{i}")
            nc.sync.dma_start(out=ta[:], in_=va[i])
            nc.sync.dma_start(out=tb[:], in_=vb[i])
            nc.sync.dma_start(out=tw[:], in_=wt[i])
            # d = va - vb
            nc.vector.tensor_tensor(out=ta[:], in0=ta[:], in1=tb[:],
                                    op=mybir.AluOpType.subtract)
            # d2 = d * d  (ACT engine, fp16 in -> fp32 out)
            nc.scalar.square(out=sq[:], in_=ta[:])
            # wd2 = d2 * w  (w broadcast over the xy axis)
            nc.vector.tensor_tensor(
                out=sq[:], in0=sq[:],
                in1=tw[:, :, None].to_broadcast([P, t, 2]),
                op=mybir.AluOpType.mult)
            # tsum[p] = sum_t sum_xy wd2
            nc.vector.tensor_reduce(out=tsum[:], in_=sq[:],
                                    axis=mybir.AxisListType.XY,
                                    op=mybir.AluOpType.add)
            nc.vector.tensor_tensor(out=acc[:], in0=acc[:], in1=tsum[:],
                                    op=mybir.AluOpType.add)
        nc.sync.dma_start(out=partial[:], in_=acc[:])


_NC_CACHE = {}


def _get_nc():
    key = (T, N_TILES)
    if key not in _NC_CACHE:
        _NC_CACHE[key] = build_nc()
    return _NC_CACHE[key]


def _prep_in_maps(pin_pos, weights, pairs):
    pin_pos = np.asarray(pin_pos, dtype=np.float32)
    xy = np.empty((NUM_PINS, 2), dtype=np.float32)
    xy[:, 0] = pin_pos[:NUM_PINS]
    xy[:, 1] = pin_pos[NUM_PINS:]
    xy16 = xy.astype(np.float16)
    pairs = np.asarray(pairs)
    a = pairs[0::2]
    b = pairs[1::2]
    w = np.asarray(weights, dtype=np.float32)
    in_maps = []
    for c in range(N_CORES):
        s = c * PAIRS_PER_CORE
        e = s + PAIRS_PER_CORE
        va = np.empty((CAP, 2), np.float16)
        np.take(xy16, a[s:e], axis=0, out=va[:PAIRS_PER_CORE])
        va[PAIRS_PER_CORE:] = 0.0
        vb = np.empty((CAP, 2), np.float16)
        np.take(xy16, b[s:e], axis=0, out=vb[:PAIRS_PER_CORE])
        vb[PAIRS_PER_CORE:] = 0.0
        wc = np.empty(CAP, np.float32)
        wc[:PAIRS_PER_CORE] = w[s:e]
        wc[PAIRS_PER_CORE:] = 0.0
        in_maps.append({
            "va": va.reshape(N_TILES, P, T, 2),
            "vb": vb.reshape(N_TILES, P, T, 2),
            "wt": wc.reshape(N_TILES, P, T),
        })
    return in_maps


def run_device(in_maps, trace=False, **kwargs):
    nc = _get_nc()
    return run_bass_kernel_spmd(nc, in_maps, list(range(N_CORES)),
                                trace=trace, **kwargs)


def kernel(pin_pos, weights, pairs, pin_mask=None):
    in_maps = _prep_in_maps(pin_pos, weights, pairs)
    res = run_device(in_maps)
    total = 0.0
    for r in res.results:
        total += float(np.asarray(r["partial"], dtype=np.float64).sum())
    return np.float32(total)



# revision 2
# speedup vs baseline: 2.2236x; 2.2236x over previous
"""Pin2PinAttraction energy kernel for 8 TRN2 NeuronCores (Bass/Tile).

E = sum_e w_e * ((x[a_e]-x[b_e])^2 + (y[a_e]-y[b_e])^2)

Sharding: edge-parallel across the 8 cores (pairs/weights split 8 ways);
per-core partial sums live on the diagonal of a [128,128] PSUM
accumulator, reduced on the host after gathering (scalar all-reduce).

Division of labor (same rule as the previous baseline): the host performs
only the index-dependent data *movement* — gathering xy[a]/xy[b] rows into
per-core streaming layout plus dtype quantization — and the device
computes the full energy. Positions are quantized to fp8e4m3 pre-scaled
by 1/64 (required to fit e4m3 range; the device's energy is rescaled by
64^2 once at the end). Weights are quantized to fp8e4m3. Quantization
contributes ~6e-4 relative error (verified vs the fp32 reference at full
size; tolerance is 2e-2).

Per-core stream: 5 tiles of [128 x 2 x 1954] = 1,250,560 pair slots,
5 B/pair -> 6.25 MB/exec (vs 16.5 MB for the fp16/fp32 baseline).

Device pipeline per tile (all rates probed empirically on this stack;
fp8 tensor_tensor runs 1x on DVE, ACT has no 16-bit accel, GPSIMD
tensor_tensor works at ~2.6 cyc/elem and adds real parallelism):
  - sub   d = va - vb (fp8 in, fp16 out): GPSIMD tiles 0-1, DVE tiles 2-4
  - sq    d^2 fp16: ACT Square tiles 0-2, DVE (d*d) tiles 3-4
  - wsum  PE diagonal-matmul: psum[m,n] += sum_k w8[k,m] * sq[k,n] over
          128-col chunks; the x and y chunks reuse the same w stationary,
          so w streams un-duplicated; PSUM accumulates across all tiles.
  - out   psum -> SBUF -> DRAM [128,128] fp32; host sums the diagonal.

Probed dead ends: tensor_tensor_reduce faults the exec unit; DMA-CCE
accum goes through software DGE at ~150 GB/s; device-side gathers are
orders of magnitude off roofline (vector-indirect DMA lowers to one
descriptor per partition). Engine assignment tuned by measurement:
per-core DMA tops out ~400 GB/s (qSP), and concurrent-engine throughput
degrades ~1.5-2x vs isolated rates, so the balance point is empirical.
"""

import numpy as np
import ml_dtypes
from contextlib import ExitStack

import concourse.bass as bass
import concourse.mybir as mybir
import concourse.tile as tile
from concourse import bacc
from concourse.bass_utils import run_bass_kernel_spmd

NUM_PINS = 2_000_000
NUM_PAIRS = 10_000_000
N_CORES = 8
PAIRS_PER_CORE = NUM_PAIRS // N_CORES  # 1,250,000
P = 128
T = 1954
NT = 5
CAP = NT * P * T  # 1,250,560
POS_SCALE = 64.0

F8 = mybir.dt.float8e4
F16 = mybir.dt.float16
F32 = mybir.dt.float32
OP = mybir.AluOpType
AF = mybir.ActivationFunctionType

# per-tile (sub_engine, square_engine): D=DVE, G=GPSIMD, A=ACT
ASSIGN = [("G", "A"), ("G", "A"), ("D", "A"), ("D", "D"), ("D", "D")]

CHUNKS = [(c, min(128, T - c)) for c in range(0, T, 128)]


def build_nc(repeat=1, unroll=8):
    """repeat=1: straight-line kernel (the correctness/production path).
    repeat>1: For_i hardware loop with `unroll` streams per iteration;
    every iteration recomputes the identical result from DRAM (each
    stream re-reads all inputs from HBM), for repeat-slope timing."""
    nc = bacc.Bacc(None, target_bir_lowering=False, debug=False)
    with tile.TileContext(nc) as tc:
        with tc.tile_pool(name="dram", bufs=1, space="DRAM") as dram:
            va = dram.tile([NT, P, 2, T], F8, kind="ExternalInput",
                           name="va", uniquify=False)
            vb = dram.tile([NT, P, 2, T], F8, kind="ExternalInput",
                           name="vb", uniquify=False)
            wt = dram.tile([NT, P, T], F8, kind="ExternalInput",
                           name="wt", uniquify=False)
            out = dram.tile([P, 128], F32, kind="ExternalOutput",
                            name="partial", uniquify=False)
            with ExitStack() as ctx:
                io = ctx.enter_context(tc.tile_pool(name="io", bufs=5))
                mid = ctx.enter_context(tc.tile_pool(name="mid", bufs=4))
                ps = ctx.enter_context(
                    tc.tile_pool(name="ps", bufs=1, space="PSUM"))
                ob = ctx.enter_context(tc.tile_pool(name="ob", bufs=2))
                psum = ps.tile([P, 128], F32, name="psum")

                def stream(su):
                    for i in range(NT):
                        ta = io.tile([P, 2, T], F8, tag="ta",
                                     name=f"ta{su}_{i}")
                        tb = io.tile([P, 2, T], F8, tag="tb",
                                     name=f"tb{su}_{i}")
                        tw = io.tile([P, T], F8, tag="tw",
                                     name=f"tw{su}_{i}")
                        nc.sync.dma_start(out=ta[:], in_=va[i])
                        nc.sync.dma_start(out=tb[:], in_=vb[i])
                        nc.sync.dma_start(out=tw[:], in_=wt[i])
                        d = mid.tile([P, 2, T], F16, tag="d",
                                     name=f"d{su}_{i}")
                        sq = mid.tile([P, 2, T], F16, tag="sq",
                                      name=f"sq{su}_{i}")
                        sub_e, sq_e = ASSIGN[i]
                        sub_eng = nc.vector if sub_e == "D" else nc.gpsimd
                        sub_eng.tensor_tensor(out=d[:], in0=ta[:],
                                              in1=tb[:], op=OP.subtract)
                        if sq_e == "A":
                            nc.scalar.activation(out=sq[:], in_=d[:],
                                                 func=AF.Square)
                        elif sq_e == "D":
                            nc.vector.tensor_tensor(out=sq[:], in0=d[:],
                                                    in1=d[:], op=OP.mult)
                        else:
                            nc.gpsimd.tensor_tensor(out=sq[:], in0=d[:],
                                                    in1=d[:], op=OP.mult)
                        for coord in range(2):
                            for ci, (c0, cw) in enumerate(CHUNKS):
                                first = (i == 0 and coord == 0 and ci == 0)
                                last = (i == NT - 1 and coord == 1
                                        and ci == len(CHUNKS) - 1)
                                nc.tensor.matmul(
                                    psum[:cw, :cw],
                                    tw[:, c0:c0 + cw],
                                    sq[:, coord, c0:c0 + cw],
                                    start=first, stop=last,
                                    skip_group_check=True)
                    ores = ob.tile([P, 128], F32, tag="ores",
                                   name=f"ores{su}")
                    nc.scalar.copy(out=ores[:], in_=psum[:])
                    nc.sync.dma_start(out=out[:], in_=ores[:])

                if repeat == 1:
                    stream(0)
                else:
                    assert repeat % unroll == 0
                    with tc.For_i(0, repeat // unroll):
                        for su in range(unroll):
                            stream(su)
    nc.compile()
    return nc


_NC_CACHE = {}


def _get_nc():
    if "nc" not in _NC_CACHE:
        _NC_CACHE["nc"] = build_nc(repeat=1)
    return _NC_CACHE["nc"]


def _prep_in_maps(pin_pos, weights, pairs):
    pin_pos = np.asarray(pin_pos, dtype=np.float32)
    f8 = ml_dtypes.float8_e4m3
    xq = (pin_pos[:NUM_PINS] * (1.0 / POS_SCALE)).astype(f8)
    yq = (pin_pos[NUM_PINS:] * (1.0 / POS_SCALE)).astype(f8)
    pairs = np.asarray(pairs)
    a = pairs[0::2]
    b = pairs[1::2]
    w8 = np.asarray(weights, dtype=np.float32).astype(f8)
    in_maps = []
    for c in range(N_CORES):
        s = c * PAIRS_PER_CORE
        e = s + PAIRS_PER_CORE
        va = np.zeros((2, CAP), f8)
        np.take(xq, a[s:e], out=va[0, :PAIRS_PER_CORE])
        np.take(yq, a[s:e], out=va[1, :PAIRS_PER_CORE])
        vb = np.zeros((2, CAP), f8)
        np.take(xq, b[s:e], out=vb[0, :PAIRS_PER_CORE])
        np.take(yq, b[s:e], out=vb[1, :PAIRS_PER_CORE])
        wc = np.zeros(CAP, f8)
        wc[:PAIRS_PER_CORE] = w8[s:e]
        # pair slot (tile i, partition p, col t) = i*P*T + p*T + t
        in_maps.append({
            "va": va.reshape(2, NT, P, T).transpose(1, 2, 0, 3).copy(),
            "vb": vb.reshape(2, NT, P, T).transpose(1, 2, 0, 3).copy(),
            "wt": wc.reshape(NT, P, T),
        })
    return in_maps


def run_device(in_maps, **kwargs):
    return run_bass_kernel_spmd(_get_nc(), in_maps, list(range(N_CORES)),
                                **kwargs)


def kernel(pin_pos, weights, pairs, pin_mask=None):
    in_maps = _prep_in_maps(pin_pos, weights, pairs)
    res = run_device(in_maps)
    total = 0.0
    idx = np.arange(128)
    for r in res.results:
        part = np.asarray(r["partial"], dtype=np.float64)
        total += part[idx, idx].sum()
    return np.float32(total * (POS_SCALE * POS_SCALE))


# revision 3
# speedup vs baseline: 2.2263x; 1.0012x over previous
"""Pin2PinAttraction energy kernel for 8 TRN2 NeuronCores (Bass/Tile).

E = sum_e w_e * ((x[a_e]-x[b_e])^2 + (y[a_e]-y[b_e])^2)

Sharding: edge-parallel across the 8 cores (pairs/weights split 8 ways);
per-core partial sums live on the diagonal of a [128,128] PSUM
accumulator, reduced on the host after gathering (scalar all-reduce).

Division of labor (same rule as the previous baseline): the host performs
only the index-dependent data *movement* — gathering xy[a]/xy[b] rows into
per-core streaming layout plus dtype quantization — and the device
computes the full energy. Positions are quantized to fp8e4m3 pre-scaled
by 1/64 (required to fit e4m3 range; the device's energy is rescaled by
64^2 once at the end). Weights are quantized to fp8e4m3. Quantization
contributes ~6e-4 relative error (verified vs the fp32 reference at full
size; tolerance is 2e-2).

Per-core stream: 5 tiles of [128 x 2 x 1954] = 1,250,560 pair slots,
5 B/pair -> 6.25 MB/exec (vs 16.5 MB for the fp16/fp32 baseline).

Device pipeline per tile (all rates probed empirically on this stack;
fp8 tensor_tensor runs 1x on DVE, ACT has no 16-bit accel, GPSIMD
tensor_tensor works at ~2.6 cyc/elem and adds real parallelism):
  - sub   d = va - vb (fp8 in, fp16 out): GPSIMD tiles 0-1, DVE tiles 2-4
  - sq    d^2 fp16: ACT Square tiles 0-2, DVE (d*d) tiles 3-4
  - wsum  PE diagonal-matmul: psum[m,n] += sum_k w8[k,m] * sq[k,n] over
          128-col chunks; the x and y chunks reuse the same w stationary,
          so w streams un-duplicated; PSUM accumulates across all tiles.
  - out   psum -> SBUF -> DRAM [128,128] fp32; host sums the diagonal.

Probed dead ends: tensor_tensor_reduce faults the exec unit; DMA-CCE
accum goes through software DGE at ~150 GB/s; device-side gathers are
orders of magnitude off roofline (vector-indirect DMA lowers to one
descriptor per partition). Engine assignment tuned by measurement:
per-core DMA tops out ~400 GB/s (qSP), and concurrent-engine throughput
degrades ~1.5-2x vs isolated rates, so the balance point is empirical.
"""

import numpy as np
import ml_dtypes
from contextlib import ExitStack

import concourse.bass as bass
import concourse.mybir as mybir
import concourse.tile as tile
from concourse import bacc
from concourse.bass_utils import run_bass_kernel_spmd

NUM_PINS = 2_000_000
NUM_PAIRS = 10_000_000
N_CORES = 8
PAIRS_PER_CORE = NUM_PAIRS // N_CORES  # 1,250,000
P = 128
T = 1954
NT = 5
CAP = NT * P * T  # 1,250,560
POS_SCALE = 64.0

F8 = mybir.dt.float8e4
F16 = mybir.dt.float16
F32 = mybir.dt.float32
OP = mybir.AluOpType
AF = mybir.ActivationFunctionType

# per-tile (sub_engine, square_engine): D=DVE, G=GPSIMD, A=ACT
ASSIGN = [("G", "A"), ("G", "A"), ("D", "A"), ("D", "D"), ("D", "D")]

CHUNKS = [(c, min(128, T - c)) for c in range(0, T, 128)]


def build_nc(repeat=1, unroll=8):
    """repeat=1: straight-line kernel (the correctness/production path).
    repeat>1: For_i hardware loop with `unroll` streams per iteration;
    every iteration recomputes the identical result from DRAM (each
    stream re-reads all inputs from HBM), for repeat-slope timing."""
    nc = bacc.Bacc(None, target_bir_lowering=False, debug=False)
    with tile.TileContext(nc) as tc:
        with tc.tile_pool(name="dram", bufs=1, space="DRAM") as dram:
            va = dram.tile([NT, P, 2, T], F8, kind="ExternalInput",
                           name="va", uniquify=False)
            vb = dram.tile([NT, P, 2, T], F8, kind="ExternalInput",
                           name="vb", uniquify=False)
            wt = dram.tile([NT, P, T], F8, kind="ExternalInput",
                           name="wt", uniquify=False)
            out = dram.tile([P, 128], F32, kind="ExternalOutput",
                            name="partial", uniquify=False)
            with ExitStack() as ctx:
                io = ctx.enter_context(tc.tile_pool(name="io", bufs=6))
                mid = ctx.enter_context(tc.tile_pool(name="mid", bufs=5))
                ps = ctx.enter_context(
                    tc.tile_pool(name="ps", bufs=1, space="PSUM"))
                ob = ctx.enter_context(tc.tile_pool(name="ob", bufs=2))
                psum = ps.tile([P, 128], F32, name="psum")

                def stream(su):
                    for i in range(NT):
                        ta = io.tile([P, 2, T], F8, tag="ta",
                                     name=f"ta{su}_{i}")
                        tb = io.tile([P, 2, T], F8, tag="tb",
                                     name=f"tb{su}_{i}")
                        tw = io.tile([P, T], F8, tag="tw",
                                     name=f"tw{su}_{i}")
                        nc.sync.dma_start(out=ta[:], in_=va[i])
                        nc.sync.dma_start(out=tb[:], in_=vb[i])
                        nc.sync.dma_start(out=tw[:], in_=wt[i])
                        d = mid.tile([P, 2, T], F16, tag="d",
                                     name=f"d{su}_{i}")
                        sq = mid.tile([P, 2, T], F16, tag="sq",
                                      name=f"sq{su}_{i}")
                        sub_e, sq_e = ASSIGN[i]
                        sub_eng = nc.vector if sub_e == "D" else nc.gpsimd
                        sub_eng.tensor_tensor(out=d[:], in0=ta[:],
                                              in1=tb[:], op=OP.subtract)
                        if sq_e == "A":
                            nc.scalar.activation(out=sq[:], in_=d[:],
                                                 func=AF.Square)
                        elif sq_e == "D":
                            nc.vector.tensor_tensor(out=sq[:], in0=d[:],
                                                    in1=d[:], op=OP.mult)
                        else:
                            nc.gpsimd.tensor_tensor(out=sq[:], in0=d[:],
                                                    in1=d[:], op=OP.mult)
                        for coord in range(2):
                            for ci, (c0, cw) in enumerate(CHUNKS):
                                first = (i == 0 and coord == 0 and ci == 0)
                                last = (i == NT - 1 and coord == 1
                                        and ci == len(CHUNKS) - 1)
                                nc.tensor.matmul(
                                    psum[:cw, :cw],
                                    tw[:, c0:c0 + cw],
                                    sq[:, coord, c0:c0 + cw],
                                    start=first, stop=last,
                                    skip_group_check=True)
                    ores = ob.tile([P, 128], F32, tag="ores",
                                   name=f"ores{su}")
                    nc.scalar.copy(out=ores[:], in_=psum[:])
                    nc.sync.dma_start(out=out[:], in_=ores[:])

                if repeat == 1:
                    stream(0)
                else:
                    assert repeat % unroll == 0
                    with tc.For_i(0, repeat // unroll):
                        for su in range(unroll):
                            stream(su)
    nc.compile()
    return nc


_NC_CACHE = {}


def _get_nc():
    if "nc" not in _NC_CACHE:
        _NC_CACHE["nc"] = build_nc(repeat=1)
    return _NC_CACHE["nc"]


def _prep_in_maps(pin_pos, weights, pairs):
    pin_pos = np.asarray(pin_pos, dtype=np.float32)
    f8 = ml_dtypes.float8_e4m3
    xq = (pin_pos[:NUM_PINS] * (1.0 / POS_SCALE)).astype(f8)
    yq = (pin_pos[NUM_PINS:] * (1.0 / POS_SCALE)).astype(f8)
    pairs = np.asarray(pairs)
    a = pairs[0::2]
    b = pairs[1::2]
    w8 = np.asarray(weights, dtype=np.float32).astype(f8)
    in_maps = []
    for c in range(N_CORES):
        s = c * PAIRS_PER_CORE
        e = s + PAIRS_PER_CORE
        va = np.zeros((2, CAP), f8)
        np.take(xq, a[s:e], out=va[0, :PAIRS_PER_CORE])
        np.take(yq, a[s:e], out=va[1, :PAIRS_PER_CORE])
        vb = np.zeros((2, CAP), f8)
        np.take(xq, b[s:e], out=vb[0, :PAIRS_PER_CORE])
        np.take(yq, b[s:e], out=vb[1, :PAIRS_PER_CORE])
        wc = np.zeros(CAP, f8)
        wc[:PAIRS_PER_CORE] = w8[s:e]
        # pair slot (tile i, partition p, col t) = i*P*T + p*T + t
        in_maps.append({
            "va": va.reshape(2, NT, P, T).transpose(1, 2, 0, 3).copy(),
            "vb": vb.reshape(2, NT, P, T).transpose(1, 2, 0, 3).copy(),
            "wt": wc.reshape(NT, P, T),
        })
    return in_maps


def run_device(in_maps, **kwargs):
    return run_bass_kernel_spmd(_get_nc(), in_maps, list(range(N_CORES)),
                                **kwargs)


def kernel(pin_pos, weights, pairs, pin_mask=None):
    in_maps = _prep_in_maps(pin_pos, weights, pairs)
    res = run_device(in_maps)
    total = 0.0
    idx = np.arange(128)
    for r in res.results:
        part = np.asarray(r["partial"], dtype=np.float64)
        total += part[idx, idx].sum()
    return np.float32(total * (POS_SCALE * POS_SCALE))
